# revision 44
# baseline (speedup 1.0000x reference)
"""Trainium2 Bass kernel for nn_CrossAttentionRouter.

Reference computation (B=2, L=4096, D=512, H=8 heads, NP=2048 queries):
    q  = LN(queries) broadcast over B            (parameter-only)
    xn = LN(x)                                   [B, L, D]
    qp = (q @ wq.T + bq) / sqrt(64)              [NP, D]  (parameter-only)
    kp = xn @ wk.T + bk                          [B, L, D]
    s_h = qp_h @ kp_h.T                          [B, H, NP, L]
    attn1 = mean_h softmax_k(s_h)                [B, NP, L]
    attn2 = softmax((log(attn1)+1e-9)/0.7)       ~ attn1^(1/0.7) normalized
    out = attn2 @ xn                             [B, NP, D] -> [B, 32, 64, D]

Device algorithm per core (8 cores, each owns 512 of the B*NP=4096 query rows,
so each core needs only its batch's x):
    phase 1: LN(x) -> xn (bf16), xbar-transpose -> xnT, kp projection -> kpT (bf16)
    phase 2, per 128-query block:
      scores per head (PE, two heads packed via tile_position row groups)
      E_h = exp(s_h) on ACT with fused row-sum z_h
      v = sum_h E_h / z_h  via PE matmuls with diag(1/z_h) stationary
        (v == H * attn1; per-row scale cancels in the final normalization)
      u = exp(ln(v)/T) on ACT with fused row-sum (u ~ attn1^(1/T) unnormalized)
      out_row = (uT.T @ xn) * (1/rowsum(u))

Host does only parameter-only prep (LayerNorm of the learnable query bank, weight
folding/transposes) plus input sharding / output gather.
"""

import numpy as np
from contextlib import ExitStack

import ml_dtypes
import orjson

import concourse.bass as bass
import concourse.tile as tile
from concourse import mybir
from concourse.bass_utils import run_bass_kernel_spmd


def _legalize_bir(bir_bytes: bytes, max_waits: int = 1) -> bytes:
    """Split multi-semaphore waits onto standalone EventSemaphore instructions.

    This walrus build accepts at most one sync-wait command per engine
    instruction; the Tile scheduler emits several. Waits gate instruction
    *issue*, so hoisting them onto preceding same-engine EventSemaphore
    instructions is semantics-preserving.
    """
    d = orjson.loads(bir_bytes)
    ctr = 0
    for fn in d["functions"]:
        for blk in fn["blocks"]:
            out = []
            for ins in blk["instructions"]:
                si = ins.get("sync_info")
                if si:
                    w = si.get("on_wait") or []
                    if len(w) > max_waits:
                        for wi in w[:-max_waits]:
                            ctr += 1
                            out.append({
                                "debug": ins.get("debug", 0),
                                "engine": ins["engine"],
                                "ins": [],
                                "name": f"I-legw{ctr}",
                                "opcode": "EventSemaphore",
                                "outs": [],
                                "sync_info": {"on_update": [],
                                              "on_wait": [wi]},
                            })
                        si["on_wait"] = w[-max_waits:]
                out.append(ins)
            blk["instructions"] = out
    return orjson.dumps(d)


def _patch_legalize(nc: "bass.Bass") -> "bass.Bass":
    orig = nc.to_json_bytes
    nc.to_json_bytes = lambda: _legalize_bir(orig())
    return nc

F32 = mybir.dt.float32
BF16 = mybir.dt.bfloat16
NP_BF16 = ml_dtypes.bfloat16
ALU = mybir.AluOpType
AF = mybir.ActivationFunctionType

B, L, D = 2, 4096, 512
H, HD = 8, 64
NQ = 32 * 64          # 2048 queries
NCORES = 8
QSH = B * NQ // NCORES  # 512 query rows per core
TEMP = 0.7
LN_EPS = 1e-5
NDB = D // 128        # 4 partition blocks of the projected dim


def _build_body(ctx: ExitStack, tc: "tile.TileContext",
                x_in, qpt_in, wkt_in, bkp_in, eye_in, out_dram,
                L_=L, QSH_=QSH):
    nc = tc.nc
    NT = L_ // 128       # l-tiles
    NQB = QSH_ // 128    # query blocks
    NQTR = L_ // 1024    # 1024-wide L quarters

    const = ctx.enter_context(tc.tile_pool(name="const", bufs=1))
    persist = ctx.enter_context(tc.tile_pool(name="persist", bufs=1))
    small = ctx.enter_context(tc.tile_pool(name="small", bufs=24))

    wkt_sb = const.tile([128, NDB * 512], BF16)    # [din_local, (dchunk, dout)]
    qpt_sb = const.tile([128, NDB * QSH_], BF16)   # [dout_local, (dblk, q)]
    bkp_sb = const.tile([128, NDB], F32)
    eye_sb = const.tile([128, 128], BF16)
    for c in range(NDB):
        nc.gpsimd.dma_start(wkt_sb[:, c * 512:(c + 1) * 512],
                          wkt_in[c * 128:(c + 1) * 128, :])
        nc.gpsimd.dma_start(qpt_sb[:, c * QSH_:(c + 1) * QSH_],
                          qpt_in[c * 128:(c + 1) * 128, :])
    nc.gpsimd.dma_start(bkp_sb[:], bkp_in[:, :])
    nc.gpsimd.dma_start(eye_sb[:], eye_in[:, :])

    xn_sb = persist.tile([128, NT * 512], BF16)    # [l_local, (ltile, d)]
    LHK = L_ // 2 if L_ >= 2048 else L_
    kpt_h = [persist.tile([128, NDB * LHK], BF16, name=f"kpt_h{i}")
             for i in range(L_ // LHK)]           # [dout_local, (dblk, l_half)]

    # ---------------- phase 1: LN(x), xnT, K projection ----------------
    with ExitStack() as p1:
        CH = min(16, NT)                   # l-tiles per x load chunk
        xstage = p1.enter_context(tc.tile_pool(name="xstage", bufs=2))
        scratch = p1.enter_context(tc.tile_pool(name="scratch", bufs=3))
        xnt_pool = p1.enter_context(tc.tile_pool(name="xnt", bufs=1))
        kp_ps_pool = p1.enter_context(
            tc.tile_pool(name="kp_ps", bufs=4, space="PSUM"))

        xnt_sb = xnt_pool.tile([128, NDB * L_], BF16)  # [din_local, (dblk, l)]

        chunks = []
        mvall = small.tile([128, 2 * NT], F32, tag="mvall", bufs=1)
        negmu_a = small.tile([128, NT], F32, tag="negmu_a", bufs=1)
        veps_a = small.tile([128, NT], F32, tag="veps_a", bufs=1)
        sd_a = small.tile([128, NT], F32, tag="sd_a", bufs=1)
        rs_a = small.tile([128, NT], F32, tag="rs_a", bufs=1)
        r0_a = small.tile([128, NT], F32, tag="r0_a", bufs=1)
        tnr_a = small.tile([128, NT], F32, tag="tnr_a", bufs=1)
        mv2 = mvall.rearrange("p (t k) -> p t k", k=2)
        LD = min(2, CH)                    # l-tiles per load piece
        for h0 in range(0, NT, CH):
            xch = xstage.tile([128, CH * 512], F32, tag="xch")
            chunks.append(xch)
            for g0 in range(0, CH, LD):
                src = x_in[(h0 + g0) * 128:(h0 + g0 + LD) * 128, :]
                src = src.rearrange("(c p) d -> p c d", p=128)
                dst = xch[:, g0 * 512:(g0 + LD) * 512]
                nc.gpsimd.dma_start(
                    dst.rearrange("p (c d) -> p c d", c=LD)[:, :, :], src)
            for tt in range(CH):
                t = h0 + tt
                xr = xch[:, tt * 512:(tt + 1) * 512]
                st6 = small.tile([128, 6], F32, tag="st6")
                nc.vector.bn_stats(st6[:], xr[:])
                nc.vector.bn_aggr(mvall[:, 2 * t:2 * t + 2], st6[:])
            hs = slice(h0, h0 + CH)
            nc.vector.tensor_scalar(negmu_a[:, hs], mv2[:, hs, 0], -1.0,
                                    None, ALU.mult)
            nc.vector.tensor_scalar(veps_a[:, hs], mv2[:, hs, 1], LN_EPS,
                                    None, ALU.add)
            nc.scalar.activation(sd_a[:, hs], veps_a[:, hs], AF.Sqrt)
            nc.vector.reciprocal(r0_a[:, hs], sd_a[:, hs])
            nc.vector.tensor_tensor(tnr_a[:, hs], r0_a[:, hs], r0_a[:, hs],
                                    ALU.mult)
            nc.vector.tensor_tensor(tnr_a[:, hs], tnr_a[:, hs], veps_a[:, hs],
                                    ALU.mult)
            nc.vector.tensor_scalar(tnr_a[:, hs], tnr_a[:, hs], -0.5, 1.5,
                                    ALU.mult, ALU.add)
            nc.vector.tensor_tensor(rs_a[:, hs], r0_a[:, hs], tnr_a[:, hs],
                                    ALU.mult)

            for t in range(h0, h0 + CH):
                xr = chunks[t // CH][:, (t % CH) * 512:((t % CH) + 1) * 512]
                # fused normalize + bf16 cast straight into xn_sb
                nc.vector.tensor_scalar(xn_sb[:, t * 512:(t + 1) * 512],
                                        xr[:], negmu_a[:, t:t + 1],
                                        rs_a[:, t:t + 1], ALU.add, ALU.mult)
            if True:
                t = h0 + CH - 1
                # one big batched block-transpose per CH l-tiles:
                # xnT block index c = t*NDB + db holds xn[t-tile, d-block db].T
                half0 = (t + 1 - CH)
                xnt_v = xnt_sb.rearrange("p (c l) -> p c l", c=NT * NDB)
                nc.sync.dma_start_transpose(
                    xnt_v[:, half0 * NDB:(t + 1) * NDB, :],
                    xn_sb[:, half0 * 512:(t + 1) * 512])
                # K-projection for this L span, overlapped with next half's LN
                xnt_4d = xnt_sb.rearrange("p (t b l) -> p t b l", t=NT, b=NDB)
                for db in range(NDB):
                    for ls in range(half0 * 128 // 1024,
                                    (t + 1) * 128 // 1024):
                        kp_ps = kp_ps_pool.tile([128, 1024], F32, tag="kp")
                        for c in range(NDB):
                            for hf in range(2):
                                t0 = (ls * 1024 + hf * 512) // 128
                                nc.tensor.matmul(
                                    kp_ps[:, hf * 512:(hf + 1) * 512],
                                    lhsT=wkt_sb[:, c * 512 + db * 128:
                                                c * 512 + (db + 1) * 128],
                                    rhs=xnt_4d[:, t0:t0 + 4, c, :],
                                    start=(c == 0), stop=(c == NDB - 1))
                        kh, lsl = (ls * 1024) // LHK, (ls * 1024) % LHK
                        nc.vector.tensor_scalar(
                            kpt_h[kh][:, db * LHK + lsl: db * LHK + lsl + 1024],
                            kp_ps[:], bkp_sb[:, db:db + 1], None, ALU.add)

    # ---------------- phase 2: attention per query block ----------------
    with ExitStack() as p2:
        sc_ps_pool = p2.enter_context(
            tc.tile_pool(name="sc_ps", bufs=2, space="PSUM"))
        v_ps_pool = p2.enter_context(
            tc.tile_pool(name="v_ps", bufs=2, space="PSUM"))
        e_pool = p2.enter_context(tc.tile_pool(name="epool", bufs=18))
        u_pool = p2.enter_context(tc.tile_pool(name="upool", bufs=2))
        ut_pool = p2.enter_context(tc.tile_pool(name="utpool", bufs=2))
        lnv_pool = p2.enter_context(tc.tile_pool(name="lnv", bufs=4))
        dg_pool = p2.enter_context(tc.tile_pool(name="dg", bufs=9))
        ostage = p2.enter_context(tc.tile_pool(name="ostage", bufs=2))
        LH = max(L_ // 2, 1024)  # E stored as per-head L-half tiles

        for qb in range(NQB):
            e_tiles = []   # per head: (half0_tile, half1_tile)
            dg_tiles = []
            for pair in range(H // 2):
                eh = [[e_pool.tile([128, LH], BF16, tag="E",
                                   name=f"E_{qb}_{pair}_{i}_{j}")
                       for j in range(max(L_ // LH, 1))] for i in range(2)]
                zp0 = small.tile([128, max(NQTR, 1)], F32, tag="zp0")
                zp1 = small.tile([128, max(NQTR, 1)], F32, tag="zp1")
                for qtr in range(NQTR):
                    half, off = (qtr * 1024) // LH, (qtr * 1024) % LH
                    s0 = sc_ps_pool.tile([128, 1024], F32, tag="s")
                    s1t = sc_ps_pool.tile([128, 1024], F32, tag="s")
                    for hf in range(2):
                        col = qtr * 1024 + hf * 512
                        nc.tensor.matmul(
                            s0[:, hf * 512:(hf + 1) * 512],
                            lhsT=qpt_sb[0:64, pair * QSH_ + qb * 128:
                                        pair * QSH_ + (qb + 1) * 128],
                            rhs=kpt_h[col // LHK][0:64, pair * LHK + col % LHK:
                                                  pair * LHK + col % LHK + 512],
                            start=True, stop=True, tile_position=(0, 0))
                        nc.tensor.matmul(
                            s1t[:, hf * 512:(hf + 1) * 512],
                            lhsT=qpt_sb[64:128, pair * QSH_ + qb * 128:
                                        pair * QSH_ + (qb + 1) * 128],
                            rhs=kpt_h[col // LHK][64:128, pair * LHK + col % LHK:
                                                  pair * LHK + col % LHK + 512],
                            start=True, stop=True, tile_position=(64, 0))
                    nc.scalar.activation(eh[0][half][:, off:off + 1024],
                                         s0[:], AF.Exp,
                                         accum_out=zp0[:, qtr:qtr + 1])
                    nc.scalar.activation(eh[1][half][:, off:off + 1024],
                                         s1t[:], AF.Exp,
                                         accum_out=zp1[:, qtr:qtr + 1])
                for e_t, zp in ((eh[0], zp0), (eh[1], zp1)):
                    z = small.tile([128, 1], F32, tag="z")
                    nc.vector.tensor_reduce(z[:], zp[:],
                                            axis=mybir.AxisListType.X, op=ALU.add)
                    w = small.tile([128, 1], F32, tag="w")
                    nc.vector.reciprocal(w[:], z[:])
                    dg = dg_pool.tile([128, 128], BF16, tag="dg")
                    nc.vector.tensor_scalar(dg[:], eye_sb[:], w[:], None, ALU.mult)
                    e_tiles.append(e_t)
                    dg_tiles.append(dg)

            us_parts = small.tile([128, max(NQTR, 1)], F32, tag="usp")
            u_t = u_pool.tile([128, L_], BF16, tag="u")
            for qtr in range(NQTR):
                half, off = (qtr * 1024) // LH, (qtr * 1024) % LH
                v_ps = v_ps_pool.tile([128, 1024], F32, tag="v")
                for h in range(H):
                    for hf in range(2):
                        nc.tensor.matmul(
                            v_ps[:, hf * 512:(hf + 1) * 512],
                            lhsT=dg_tiles[h][:],
                            rhs=e_tiles[h][half][:, off + hf * 512:
                                                 off + (hf + 1) * 512],
                            start=(h == 0), stop=(h == H - 1))
                lnv = lnv_pool.tile([128, 1024], F32, tag="lnv")
                nc.scalar.activation(lnv[:], v_ps[:], AF.Ln)
                nc.scalar.activation(u_t[:, qtr * 1024:(qtr + 1) * 1024],
                                     lnv[:], AF.Exp, scale=1.0 / TEMP,
                                     accum_out=us_parts[:, qtr:qtr + 1])
            us = small.tile([128, 1], F32, tag="us")
            nc.vector.tensor_reduce(us[:], us_parts[:],
                                    axis=mybir.AxisListType.X, op=ALU.add)
            rus = small.tile([128, 1], F32, tag="rus")
            nc.vector.reciprocal(rus[:], us[:])

            ut_t = ut_pool.tile([128, L_], BF16, tag="uT")
            ut_v = ut_t.rearrange("p (c l) -> p c l", c=L_ // 128)
            out_ps = v_ps_pool.tile([128, 512], F32, tag="v")
            HL = min(1024, L_)
            for lo in range(0, L_, HL):
                nc.sync.dma_start_transpose(
                    ut_v[:, lo // 128:(lo + HL) // 128, :],
                    u_t[:, lo:lo + HL])
                for c in range(lo // 128, (lo + HL) // 128):
                    nc.tensor.matmul(out_ps[:],
                                     lhsT=ut_t[:, c * 128:(c + 1) * 128],
                                     rhs=xn_sb[:, c * 512:(c + 1) * 512],
                                     start=(c == 0), stop=(c == L_ // 128 - 1))
            outf = ostage.tile([128, 512], F32, tag="outf")
            nc.vector.tensor_scalar(outf[:], out_ps[:], rus[:], None, ALU.mult)
            nc.gpsimd.dma_start(out_dram[qb * 128:(qb + 1) * 128, :], outf[:])


def build_nc(L_=L, QSH_=QSH):
    nc = bass.Bass()
    x_in = nc.declare_dram_parameter("x_b", [L_, D], F32, isOutput=False)
    qpt_in = nc.declare_dram_parameter("qpt", [D, QSH_], BF16, isOutput=False)
    wkt_in = nc.declare_dram_parameter("wkt", [D, D], BF16, isOutput=False)
    bkp_in = nc.declare_dram_parameter("bkp", [128, NDB], F32, isOutput=False)
    eye_in = nc.declare_dram_parameter("eye", [128, 128], BF16, isOutput=False)
    out_dram = nc.declare_dram_parameter("out", [QSH_, D], F32, isOutput=True)
    with ExitStack() as ctx:
        tc = ctx.enter_context(tile.TileContext(nc))
        _build_body(ctx, tc, x_in, qpt_in, wkt_in, bkp_in, eye_in, out_dram,
                    L_=L_, QSH_=QSH_)
    return _patch_legalize(nc)


def host_prep(x, queries, wq, wk, bq, bk, gamma_q, beta_q, gamma_x, beta_x,
              L_=L, QSH_=QSH, ncores=NCORES):
    """Parameter-only host prep + per-core input maps."""
    x = np.asarray(x, np.float32)
    queries = np.asarray(queries, np.float32)
    wq = np.asarray(wq, np.float32)
    wk = np.asarray(wk, np.float32)
    bq = np.asarray(bq, np.float32)
    bk = np.asarray(bk, np.float32)
    gamma_q = np.asarray(gamma_q, np.float32)
    beta_q = np.asarray(beta_q, np.float32)
    gamma_x = np.asarray(gamma_x, np.float32)
    beta_x = np.asarray(beta_x, np.float32)

    # fold LN affines into the projections (exact):
    #   kp = (LN0(x)*gx + bx) @ wk.T + bk = LN0(x) @ (wk*gx).T + (wk@bx + bk)
    wq_f = wq * gamma_q[None, :]
    bq_f = wq @ beta_q + bq
    wk_f = wk * gamma_x[None, :]
    bk_f = wk @ beta_x + bk

    # parameter-only query path
    qflat = queries.reshape(NQ, D)
    mu = qflat.mean(-1, keepdims=True)
    var = ((qflat - mu) ** 2).mean(-1, keepdims=True)
    qn = (qflat - mu) / np.sqrt(var + LN_EPS)
    qp = (qn @ wq_f.T + bq_f) * np.float32(1.0 / np.sqrt(HD))  # [NQ, D]

    nqb_total = B * NQ // QSH_  # shards across batches*queries
    per_batch = nqb_total // B
    in_maps = []
    wkt_np = np.ascontiguousarray(wk_f.T).astype(NP_BF16)
    bkp_np = np.ascontiguousarray(bk_f.reshape(NDB, 128).T).astype(np.float32)
    eye_np = np.eye(128, dtype=NP_BF16)
    for c in range(ncores):
        b = c // per_batch
        q0 = (c % per_batch) * QSH_
        in_maps.append(dict(
            x_b=np.ascontiguousarray(x[b, :L_, :]),
            qpt=np.ascontiguousarray(qp[q0:q0 + QSH_].T).astype(NP_BF16),
            wkt=wkt_np,
            bkp=bkp_np,
            eye=eye_np,
        ))
    return in_maps, (gamma_x, beta_x)


_NC_CACHE = {}


def _get_nc(L_=L, QSH_=QSH):
    key = (L_, QSH_)
    if key not in _NC_CACHE:
        _NC_CACHE[key] = build_nc(L_, QSH_)
    return _NC_CACHE[key]


def run_sharded(inputs, trace=False):
    in_maps, (gamma_x, beta_x) = host_prep(**inputs)
    nc = _get_nc()
    res = run_bass_kernel_spmd(nc, in_maps, list(range(NCORES)), trace=trace)
    outs = [res.results[c]["out"] for c in range(NCORES)]
    out = np.concatenate(outs, axis=0).reshape(B, NQ, D)
    if not (np.allclose(gamma_x, 1.0) and np.allclose(beta_x, 0.0)):
        out = out * gamma_x[None, None, :] + beta_x[None, None, :]
    return out.reshape(B, 32, 64, D).astype(np.float32), res


def kernel(**inputs):
    out, _ = run_sharded(inputs, trace=False)
    return out
